# revision 31
# baseline (speedup 1.0000x reference)
"""Trainium2 Bass kernel for AdvancedKANLayer (v2).

Math (per reference):
  xn    = LayerNorm(x) * ln_w + ln_b           (eps=1e-5)
  base  = silu(xn) @ base_weight.T             [B,S,O]
  t     = tanh(xn)
  basis = cos(pi*k*t), k=1..8
  spl   = einsum('bsig,oig->bso', basis, spline_weight)
  out   = base + spl

Strategy: data-parallel over batch (8 cores, one batch entry each, no
collectives).  Per core the whole thing is one K=18432 GEMM:
  out[o, t] = sum_k W_all[k, o] * panel[k, t]
where panel rows are [silu(xn); cos(1*pi*t); ...; cos(8*pi*t)] per
I-chunk, generated on-chip via a Chebyshev ladder from
c1 = cos(pi*t) = 1 - 2*sin(pi*t/2)^2 (ScalarE Sin valid on [-pi,pi]).

v2 changes vs v1:
 - x arrives HOST-TRANSPOSED as xt [I, T]: panel generation reads
   i-major tiles straight from DRAM; the 256 PE transposes are gone.
 - LN stats per token via fp32 ones-matmuls on TensorE (column sums of
   x and x^2 accumulated over the 16 i-blocks into one PSUM bank at
   partitions 0/32), tiny row math on DVE, then one gpsimd
   partition_broadcast of [istd | -mu*istd] rows.  Normalization is two
   DVE tensor_tensor ops with free-dim-broadcast APs.
 - k-step order interleaves i-block PAIRS (s = pair*18 + 2m + sub) so
   tanh/silu/sin run at [128,1024] (half the ACT dispatch overhead);
   panel tiles are [128,1024] pair-tiles, matmuls consume 512-halves.
 - KG=8 weight groups (2KB DMA lines, half the issue count), wt bf16.
 - Output written bf16 (host upcasts): half the out-DMA bytes.
 - n_race=5 o-tiles race panel generation; ladder ops split between
   ACT and DVE to balance both near ~120us/chunk.
"""

import math

import numpy as np
import ml_dtypes

import concourse.bass as bass
import concourse.mybir as mybir
import concourse.tile as tile
from concourse import bacc
from concourse.bass import ds, ts
from concourse.bass_utils import run_bass_kernel_spmd

F32 = mybir.dt.float32
BF16 = mybir.dt.bfloat16
FP8E5 = mybir.dt.float8e5
PM = mybir.MatmulPerfMode
AF = mybir.ActivationFunctionType
ALU = mybir.AluOpType

EPS = 1e-5

# geometry (full problem, per core)
B = 8
T = 2048          # tokens per core (= S, one batch entry per core)
I = 2048          # input dim
O = 2048          # output dim
G = 8             # cos harmonics
TCH = 512         # token chunk (matmul N)
NCH = T // TCH    # 4
NIC = I // 128    # 16 I-chunks
NP = NIC // 2     # 8 pairs
NM = G + 1        # 9 panel row-groups per ic (silu + 8 cos)
NKB = NIC * G     # 128 bf16 (cos) k-steps of 128; silu goes fp8 DoubleRow
KG = 8            # k-steps per weight DMA group
NG = NKB // KG    # 16
NOT = O // 128    # 16 o-tiles
N_RACE = 4        # o-tiles racing panel generation


def build_nc(affine=False):
    nc = bacc.Bacc("TRN2", target_bir_lowering=False, debug=False)
    xt_ext = nc.declare_dram_parameter("xt", [I, T], F32, isOutput=False)
    lnw_ext = nc.declare_dram_parameter("lnw", [I], F32, isOutput=False)
    lnb_ext = nc.declare_dram_parameter("lnb", [I], F32, isOutput=False)
    wt_ext = nc.declare_dram_parameter("wt", [NOT, NG, 128, KG, 128], BF16, isOutput=False)
    wt0_ext = nc.declare_dram_parameter("wt0", [NOT, 128, NP, 2, 128], FP8E5, isOutput=False)
    out_ext = nc.declare_dram_parameter("out", [O, T], BF16, isOutput=True)

    with tile.TileContext(nc) as tc:
        with (
            tc.tile_pool(name="consts", bufs=1) as consts,
            tc.tile_pool(name="xsp", bufs=2) as xsp,       # stats x stream [128,512] f32
            tc.tile_pool(name="sqp", bufs=2) as sqp,       # squares [128,512] f32
            tc.tile_pool(name="xgp", bufs=3) as xgp,       # gen pair tiles + tanh scratch [128,1024] f32
            tc.tile_pool(name="shp", bufs=1) as shp,       # sin tile [128,1024] f32
            tc.tile_pool(name="scrp", bufs=3) as scrp,     # ladder scratch [128,512] f32
            tc.tile_pool(name="ladp", bufs=1) as ladp,     # c1..c4 [128,512] f32
            tc.tile_pool(name="rowp", bufs=1) as rowp,     # stat rows [1,*] f32
            tc.tile_pool(name="bcp", bufs=1) as bcp,       # broadcast [128,1024] f32
            tc.tile_pool(name="panelp", bufs=1) as panelp, # 72 pair-tiles [128,1024] bf16
            tc.tile_pool(name="wp", bufs=7) as wp,         # weights [128,KG,128] bf16
            tc.tile_pool(name="wdrp", bufs=2) as wdrp,     # DR silu weights [128,NP,2,128] fp8
            tc.tile_pool(name="stgp", bufs=2) as stgp,     # out staging [128,512] bf16
            tc.tile_pool(name="statps", bufs=1, space="PSUM") as statps,
            tc.tile_pool(name="mmps", bufs=7, space="PSUM") as mmps,
        ):
            eps_sb = consts.tile([1, 1], F32)
            nc.gpsimd.memset(eps_sb[:], EPS)
            ones_sb = consts.tile([128, 1], F32)
            nc.gpsimd.memset(ones_sb[:], 1.0)
            neg1_sb = consts.tile([128, 1], F32)
            nc.gpsimd.memset(neg1_sb[:], -1.0)
            ones_bf = consts.tile([128, 1], BF16)
            nc.gpsimd.memset(ones_bf[:], 1.0)
            if affine:
                lnw_sb = consts.tile([128, NIC], F32)
                nc.sync.dma_start(lnw_sb[:], lnw_ext.rearrange("(f p) -> p f", p=128))
                lnb_sb = consts.tile([128, NIC], F32)
                nc.sync.dma_start(lnb_sb[:], lnb_ext.rearrange("(f p) -> p f", p=128))

            ptiles = {}     # (pair, m) -> [128,1024] bf16 pair-tile
            bc_map = {}     # chunk -> broadcast tile
            wgq = {}        # (ot, g) -> prefetched weight tile

            def rhs(sp, q=None):
                """bf16 cos k-step sp in [0,128): pair-major, m=1..8."""
                pair, r = divmod(sp, 2 * G)
                m, sub = divmod(r, 2)
                m += 1
                if q is None:
                    return ptiles[(pair, m)][:, ds(sub * TCH, TCH)]
                return ptiles[(pair, m)][:, ds(sub * TCH + q * 128, 128)]

            def wg_get(ot, g):
                key = (ot, g)
                if key in wgq:
                    return wgq.pop(key)
                w_ = wp.tile([128, KG, 128], BF16, tag="wg")
                nc.sync.dma_start(w_[:], wt_ext[ot, g])
                return w_

            def wg_prefetch(ot, g):
                w_ = wp.tile([128, KG, 128], BF16, tag="wg")
                nc.sync.dma_start(w_[:], wt_ext[ot, g])
                wgq[(ot, g)] = w_

            def stats_steps(c):
                """Closures: 16 per-ic steps + 1 rows/broadcast step."""
                stp = statps.tile([128, TCH], F32, tag="st", name=f"st{c}")

                def ic_step(ic):
                    def f():
                        xs = xsp.tile([128, TCH], F32, tag="xs")
                        nc.gpsimd.dma_start(
                            xs[:], xt_ext[ds(ic * 128, 128), ds(c * TCH, TCH)]
                        )
                        xb = xsp.tile([128, TCH], BF16, tag="xb")
                        nc.scalar.copy(xb[:], xs[:])
                        sq = sqp.tile([128, TCH], BF16, tag="sq")
                        nc.scalar.square(sq[:], xs[:])
                        nc.tensor.matmul(stp[0:1, :], ones_bf[:], xb[:],
                                         start=(ic == 0), stop=(ic == NIC - 1))
                        nc.tensor.matmul(stp[32:33, :], ones_bf[:], sq[:],
                                         start=(ic == 0), stop=(ic == NIC - 1))
                    return f

                def rows_step():
                    mean = rowp.tile([1, TCH], F32, tag="mean", name=f"mean{c}")
                    nc.vector.tensor_scalar_mul(mean[:], stp[0:1, :], 1.0 / I)
                    var = rowp.tile([1, TCH], F32, tag="var", name=f"var{c}")
                    nc.vector.tensor_scalar_mul(var[:], stp[32:33, :], 1.0 / I)
                    row = rowp.tile([1, 2 * TCH], F32, tag="row", name=f"row{c}")
                    rr = row[0:1, 0:TCH]
                    nc.vector.tensor_tensor(rr, mean[:], mean[:], ALU.mult)
                    nc.vector.tensor_sub(var[:], var[:], rr)
                    # std in place of var, istd into the row's first half
                    nc.scalar.activation(var[:], var[:], AF.Sqrt, bias=eps_sb[:])
                    nc.vector.reciprocal(rr, var[:])
                    nc.vector.scalar_tensor_tensor(
                        row[0:1, TCH:2 * TCH], mean[:], -1.0, rr,
                        ALU.mult, ALU.mult,
                    )
                    bc = bcp.tile([128, 2 * TCH], F32, tag="bc", name=f"bc{c}")
                    nc.gpsimd.partition_broadcast(bc[:], row[0:1, :])
                    bc_map[c] = bc

                return [ic_step(ic) for ic in range(NIC)] + [rows_step]

            def gen_chunk(c, deferred=None):
                """Panel generation for chunk c; race matmuls interleaved.
                `deferred` emits the previous chunk's last o-tile drain
                after pair 0 (keeps DVE free to pre-generate pair 0)."""
                bc = bc_map.pop(c)
                iv = bc[:, 0:TCH].unsqueeze(1).broadcast_to((128, 2, TCH))
                nv = bc[:, TCH:2 * TCH].unsqueeze(1).broadcast_to((128, 2, TCH))
                pss = [
                    mmps.tile([128, TCH], F32, tag="ps", name=f"rps{r}_{c}")
                    for r in range(N_RACE)
                ]
                g_next = 0

                def race_mm(g_hi):
                    nonlocal g_next
                    for g in range(g_next, g_hi):
                        for r in range(N_RACE):
                            w_ = wg_get(r, g)
                            for ks in range(KG):
                                sp = g * KG + ks
                                nc.tensor.matmul(
                                    pss[r][:], w_[:, ks, :], rhs(sp),
                                    start=(sp == 0), stop=False,
                                )
                    g_next = g_hi

                for pair in range(NP):
                    xg = xgp.tile([128, 2 * TCH], F32, tag="xg",
                                  name=f"xg_{c}_{pair}")
                    for sub in range(2):
                        ic = 2 * pair + sub
                        nc.gpsimd.dma_start(
                            xg[:, ds(sub * TCH, TCH)],
                            xt_ext[ds(ic * 128, 128), ds(c * TCH, TCH)],
                        )
                    # normalize in place: xn = x*istd + (-mu*istd), per-token
                    nc.vector.tensor_tensor(xg[:], xg[:], iv, ALU.mult)
                    nc.vector.tensor_tensor(xg[:], xg[:], nv, ALU.add)
                    if affine:
                        for sub in range(2):
                            ic = 2 * pair + sub
                            nc.scalar.activation(
                                xg[:, ds(sub * TCH, TCH)], xg[:, ds(sub * TCH, TCH)],
                                AF.Identity,
                                bias=lnb_sb[:, ic:ic + 1], scale=lnw_sb[:, ic:ic + 1],
                            )

                    def pt(m):
                        t_ = panelp.tile([128, 2 * TCH], BF16, tag=f"p{pair}_{m}",
                                         name=f"pan_{c}_{pair}_{m}")
                        ptiles[(pair, m)] = t_
                        return t_

                    th = xgp.tile([128, 2 * TCH], F32, tag="xg",
                                  name=f"th_{c}_{pair}")
                    nc.scalar.activation(th[:], xg[:], AF.Tanh)
                    p0 = panelp.tile([128, 2, TCH], FP8E5, tag=f"p{pair}_0",
                                     name=f"pan_{c}_{pair}_0")
                    ptiles[(pair, 0)] = p0
                    for sub in range(2):
                        nc.scalar.activation(p0[:, sub, :], xg[:, ds(sub * TCH, TCH)],
                                             AF.Silu)
                    sh = shp.tile([128, 2 * TCH], F32, tag="sh")
                    nc.scalar.activation(sh[:], th[:], AF.Sin, scale=math.pi / 2)

                    p1, p2, p3, p4 = pt(1), pt(2), pt(3), pt(4)
                    p5, p6, p7, p8 = pt(5), pt(6), pt(7), pt(8)
                    for sub in range(2):
                        hs = ds(sub * TCH, TCH)
                        shh = sh[:, hs]

                        def scr(tag_i):
                            return scrp.tile([128, TCH], F32, tag="scr",
                                             name=f"scr{tag_i}_{c}_{pair}_{sub}")

                        def lad(tag):
                            return ladp.tile([128, TCH], F32, tag=tag,
                                             name=f"lad_{tag}_{c}_{pair}_{sub}")

                        # u = -2*sh^2 ; c1 = u + 1  (c1 on ACT)
                        u = scr("u")
                        nc.vector.scalar_tensor_tensor(u[:], shh, -2.0, shh,
                                                       ALU.mult, ALU.mult)
                        c1 = lad("c1")
                        nc.vector.tensor_scalar_add(c1[:], u[:], 1.0)
                        sq1 = scr("s1")
                        nc.scalar.square(sq1[:], c1[:])
                        c2 = lad("c2")
                        nc.vector.tensor_scalar(c2[:], sq1[:], 2.0, -1.0,
                                                ALU.mult, ALU.add)
                        u3 = scr("u3")
                        nc.vector.scalar_tensor_tensor(u3[:], c2[:], 2.0, c1[:],
                                                       ALU.mult, ALU.mult)
                        c3 = lad("c3")
                        nc.vector.tensor_sub(c3[:], u3[:], c1[:])
                        sq2 = scr("s2")
                        nc.scalar.square(sq2[:], c2[:])
                        c4 = lad("c4")
                        nc.vector.tensor_scalar(c4[:], sq2[:], 2.0, -1.0,
                                                ALU.mult, ALU.add)
                        # exports m=1..4
                        nc.scalar.copy(p1[:, hs], c1[:])
                        nc.scalar.copy(p2[:, hs], c2[:])
                        nc.scalar.copy(p3[:, hs], c3[:])
                        nc.vector.tensor_copy(p4[:, hs], c4[:])
                        # leaves m=5..8 straight to panel halves
                        u5 = scr("u5")
                        nc.vector.scalar_tensor_tensor(u5[:], c3[:], 2.0, c2[:],
                                                       ALU.mult, ALU.mult)
                        nc.vector.tensor_sub(p5[:, hs], u5[:], c1[:])
                        sq3 = scr("s3")
                        nc.scalar.square(sq3[:], c3[:])
                        nc.scalar.activation(p6[:, hs], sq3[:], AF.Identity,
                                             bias=neg1_sb[:], scale=2.0)
                        u7 = scr("u7")
                        nc.vector.scalar_tensor_tensor(u7[:], c4[:], 2.0, c3[:],
                                                       ALU.mult, ALU.mult)
                        nc.vector.tensor_sub(p7[:, hs], u7[:], c1[:])
                        sq4 = scr("s4")
                        nc.scalar.square(sq4[:], c4[:])
                        nc.scalar.activation(p8[:, hs], sq4[:], AF.Identity,
                                             bias=neg1_sb[:], scale=2.0)

                    if pair == 0 and deferred is not None:
                        deferred()
                    race_mm(2 * (pair + 1))
                race_mm(NG)
                # silu fp8 DoubleRow burst closes each race accumulation
                for r in range(N_RACE):
                    wdr = wdrp.tile([128, NP, 2, 128], FP8E5, tag="wdr",
                                    name=f"rwdr{r}_{c}")
                    nc.sync.dma_start(wdr[:], wt0_ext[r])
                    for pair in range(NP):
                        nc.tensor.matmul(
                            pss[r][:], wdr[:, pair, :, :], ptiles[(pair, 0)][:],
                            start=False, stop=(pair == NP - 1),
                            perf_mode=PM.DoubleRow, skip_group_check=True,
                        )
                for r in range(N_RACE):
                    stg = stgp.tile([128, TCH], BF16, tag="stg",
                                    name=f"rstg{r}_{c}")
                    nc.vector.tensor_copy(stg[:], pss[r][:])
                    nc.gpsimd.dma_start(
                        out_ext[ds(r * 128, 128), ds(c * TCH, TCH)], stg[:]
                    )

            def mm_chunk(c, steps):
                """O-tiles N_RACE..15; silu DR matmuls open each pass;
                next-chunk stats steps injected spread over early
                o-tiles.  Returns a deferred closure for the last
                o-tile's drain (or None)."""
                n_ots = NOT - N_RACE
                deferred = None
                for oi, ot in enumerate(range(N_RACE, NOT)):
                    last = (c == NCH - 1) and (ot == NOT - 1)
                    ps = mmps.tile([128, TCH], F32, tag="ps", name=f"mps{c}_{ot}")
                    wdr = wdrp.tile([128, NP, 2, 128], FP8E5, tag="wdr",
                                    name=f"wdr{c}_{ot}")
                    nc.sync.dma_start(wdr[:], wt0_ext[ot])
                    for pair in range(NP):
                        nc.tensor.matmul(
                            ps[:], wdr[:, pair, :, :], ptiles[(pair, 0)][:],
                            start=(pair == 0), stop=False,
                            perf_mode=PM.DoubleRow, skip_group_check=True,
                        )
                    for g in range(NG):
                        if steps and (g % 3 == 1):
                            steps.pop(0)()
                        w_ = wg_get(ot, g)
                        for ks in range(KG):
                            sp = g * KG + ks
                            if last and sp == NKB - 1:
                                for q in range(4):
                                    nc.tensor.matmul(
                                        ps[:, ds(q * 128, 128)], w_[:, ks, :],
                                        rhs(sp, q), start=False, stop=True,
                                        skip_group_check=True,
                                    )
                            else:
                                nc.tensor.matmul(
                                    ps[:], w_[:, ks, :], rhs(sp),
                                    start=False, stop=(sp == NKB - 1),
                                )

                    def drain(ps=ps, ot=ot):
                        stg = stgp.tile([128, TCH], BF16, tag="stg",
                                        name=f"stg{c}_{ot}")
                        nc.vector.tensor_copy(stg[:], ps[:])
                        nc.gpsimd.dma_start(
                            out_ext[ds(ot * 128, 128), ds(c * TCH, TCH)], stg[:]
                        )

                    if last:
                        # pipelined sliced drain to shorten the kernel tail
                        stg = stgp.tile([128, TCH], BF16, tag="stg",
                                        name=f"stg{c}_{ot}")
                        for q in range(4):
                            sl = ds(q * 128, 128)
                            nc.vector.tensor_copy(stg[:, sl], ps[:, sl])
                            nc.gpsimd.dma_start(
                                out_ext[ds(ot * 128, 128),
                                        ds(c * TCH + q * 128, 128)],
                                stg[:, sl],
                            )
                    elif oi == n_ots - 1 and c + 1 < NCH:
                        deferred = drain
                    else:
                        drain()
                while steps:
                    steps.pop(0)()
                return deferred

            # --- program ---
            for g in range(2):
                for r in range(N_RACE):
                    if len(wgq) < 7:
                        wg_prefetch(r, g)
            for f in stats_steps(0):
                f()
            deferred = None
            for c in range(NCH):
                gen_chunk(c, deferred=deferred)
                steps = stats_steps(c + 1) if c + 1 < NCH else []
                deferred = mm_chunk(c, steps)

    nc.compile()
    return nc


def prep_weights(base_weight, spline_weight):
    """Host-side weight prep.
    wt  (bf16 cos part): k-step sp = pair*16 + (m-1)*2 + sub, tiled
        [ot, g, k_in, ks, o_in] for [128, KG, 128] DMAs.
    wt0 (fp8e5 silu part, DoubleRow): [ot, k_in, pair, sub, o_in]."""
    w = np.empty((G, I, O), np.float32)
    for g in range(G):
        w[g] = spline_weight[:, :, g].T       # [i, o]
    w = w.reshape(G, NP, 2, 128, O).transpose(1, 0, 2, 3, 4)  # [pair, m-1, sub, 128, o]
    w = w.reshape(NKB * 128, O)
    w = w.reshape(NG, KG, 128, NOT, 128).transpose(3, 0, 2, 1, 4)
    wt = np.ascontiguousarray(w.astype(ml_dtypes.bfloat16))
    wb = base_weight.T.reshape(NP, 2, 128, NOT, 128)  # [pair, sub, kin, ot, oin]
    wb = wb.transpose(3, 2, 0, 1, 4)                  # [ot, kin, pair, sub, oin]
    wt0 = np.ascontiguousarray(wb.astype(ml_dtypes.float8_e5m2))
    return wt, wt0


_NC_CACHE = {}


def _get_nc(affine=False):
    if affine not in _NC_CACHE:
        _NC_CACHE[affine] = build_nc(affine=affine)
    return _NC_CACHE[affine]


def kernel(x, ln_weight, ln_bias, base_weight, spline_weight):
    x = np.asarray(x, np.float32)
    ln_weight = np.asarray(ln_weight, np.float32)
    ln_bias = np.asarray(ln_bias, np.float32)
    affine = not (np.all(ln_weight == 1.0) and np.all(ln_bias == 0.0))
    wt, wt0 = prep_weights(np.asarray(base_weight, np.float32),
                           np.asarray(spline_weight, np.float32))
    nc = _get_nc(affine)
    in_maps = [
        {
            "xt": np.ascontiguousarray(x[b].T),
            "lnw": ln_weight,
            "lnb": ln_bias,
            "wt": wt,
            "wt0": wt0,
        }
        for b in range(B)
    ]
    res = run_bass_kernel_spmd(nc, in_maps, core_ids=list(range(B)))
    out = np.stack([res.results[b]["out"].astype(np.float32).T for b in range(B)])
    return np.ascontiguousarray(out)


# revision 34
# speedup vs baseline: 1.0158x; 1.0158x over previous
"""Trainium2 Bass kernel for AdvancedKANLayer (v2).

Math (per reference):
  xn    = LayerNorm(x) * ln_w + ln_b           (eps=1e-5)
  base  = silu(xn) @ base_weight.T             [B,S,O]
  t     = tanh(xn)
  basis = cos(pi*k*t), k=1..8
  spl   = einsum('bsig,oig->bso', basis, spline_weight)
  out   = base + spl

Strategy: data-parallel over batch (8 cores, one batch entry each, no
collectives).  Per core the whole thing is one K=18432 GEMM:
  out[o, t] = sum_k W_all[k, o] * panel[k, t]
where panel rows are [silu(xn); cos(1*pi*t); ...; cos(8*pi*t)] per
I-chunk, generated on-chip via a Chebyshev ladder from
c1 = cos(pi*t) = 1 - 2*sin(pi*t/2)^2 (ScalarE Sin valid on [-pi,pi]).

v2 changes vs v1:
 - x arrives HOST-TRANSPOSED as xt [I, T]: panel generation reads
   i-major tiles straight from DRAM; the 256 PE transposes are gone.
 - LN stats per token via fp32 ones-matmuls on TensorE (column sums of
   x and x^2 accumulated over the 16 i-blocks into one PSUM bank at
   partitions 0/32), tiny row math on DVE, then one gpsimd
   partition_broadcast of [istd | -mu*istd] rows.  Normalization is two
   DVE tensor_tensor ops with free-dim-broadcast APs.
 - k-step order interleaves i-block PAIRS (s = pair*18 + 2m + sub) so
   tanh/silu/sin run at [128,1024] (half the ACT dispatch overhead);
   panel tiles are [128,1024] pair-tiles, matmuls consume 512-halves.
 - KG=8 weight groups (2KB DMA lines, half the issue count), wt bf16.
 - Output written bf16 (host upcasts): half the out-DMA bytes.
 - n_race=5 o-tiles race panel generation; ladder ops split between
   ACT and DVE to balance both near ~120us/chunk.
"""

import math

import numpy as np
import ml_dtypes

import concourse.bass as bass
import concourse.mybir as mybir
import concourse.tile as tile
from concourse import bacc
from concourse.bass import ds, ts
from concourse.bass_utils import run_bass_kernel_spmd

F32 = mybir.dt.float32
BF16 = mybir.dt.bfloat16
FP8E5 = mybir.dt.float8e5
PM = mybir.MatmulPerfMode
AF = mybir.ActivationFunctionType
ALU = mybir.AluOpType

EPS = 1e-5

# geometry (full problem, per core)
B = 8
T = 2048          # tokens per core (= S, one batch entry per core)
I = 2048          # input dim
O = 2048          # output dim
G = 8             # cos harmonics
TCH = 512         # token chunk (matmul N)
NCH = T // TCH    # 4
NIC = I // 128    # 16 I-chunks
NP = NIC // 2     # 8 pairs
NM = G + 1        # 9 panel row-groups per ic (silu + 8 cos)
NKB = NIC * G     # 128 bf16 (cos) k-steps of 128; silu goes fp8 DoubleRow
KG = 8            # k-steps per weight DMA group
NG = NKB // KG    # 16
NOT = O // 128    # 16 o-tiles
N_RACE = 5        # o-tiles racing panel generation


def build_nc(affine=False):
    nc = bacc.Bacc("TRN2", target_bir_lowering=False, debug=False)
    xt_ext = nc.declare_dram_parameter("xt", [I, T], F32, isOutput=False)
    lnw_ext = nc.declare_dram_parameter("lnw", [I], F32, isOutput=False)
    lnb_ext = nc.declare_dram_parameter("lnb", [I], F32, isOutput=False)
    wt_ext = nc.declare_dram_parameter("wt", [NOT, NG, 128, KG, 128], BF16, isOutput=False)
    wt0_ext = nc.declare_dram_parameter("wt0", [NOT, 128, NP, 2, 128], FP8E5, isOutput=False)
    out_ext = nc.declare_dram_parameter("out", [O, T], BF16, isOutput=True)

    with tile.TileContext(nc) as tc:
        with (
            tc.tile_pool(name="consts", bufs=1) as consts,
            tc.tile_pool(name="xsp", bufs=2) as xsp,       # stats x stream [128,512] f32
            tc.tile_pool(name="sqp", bufs=2) as sqp,       # squares [128,512] f32
            tc.tile_pool(name="xgp", bufs=3) as xgp,       # gen pair tiles + tanh scratch [128,1024] f32
            tc.tile_pool(name="shp", bufs=1) as shp,       # sin tile [128,1024] f32
            tc.tile_pool(name="scrp", bufs=3) as scrp,     # ladder scratch [128,512] f32
            tc.tile_pool(name="ladp", bufs=1) as ladp,     # c1..c4 [128,512] f32
            tc.tile_pool(name="rowp", bufs=1) as rowp,     # stat rows [1,*] f32
            tc.tile_pool(name="bcp", bufs=1) as bcp,       # broadcast [128,1024] f32
            tc.tile_pool(name="panelp", bufs=NP) as panelp,  # 9 tags x 8 pair-tiles
            tc.tile_pool(name="wp", bufs=7) as wp,         # weights [128,KG,128] bf16
            tc.tile_pool(name="wdrp", bufs=2) as wdrp,     # DR silu weights [128,NP,2,128] fp8
            tc.tile_pool(name="stgp", bufs=2) as stgp,     # out staging [128,512] bf16
            tc.tile_pool(name="statps", bufs=1, space="PSUM") as statps,
            tc.tile_pool(name="mmps", bufs=7, space="PSUM") as mmps,
        ):
            eps_sb = consts.tile([1, 1], F32)
            nc.gpsimd.memset(eps_sb[:], EPS)
            ones_sb = consts.tile([128, 1], F32)
            nc.gpsimd.memset(ones_sb[:], 1.0)
            neg1_sb = consts.tile([128, 1], F32)
            nc.gpsimd.memset(neg1_sb[:], -1.0)
            ones_bf = consts.tile([128, 1], BF16)
            nc.gpsimd.memset(ones_bf[:], 1.0)
            if affine:
                lnw_sb = consts.tile([128, NIC], F32)
                nc.sync.dma_start(lnw_sb[:], lnw_ext.rearrange("(f p) -> p f", p=128))
                lnb_sb = consts.tile([128, NIC], F32)
                nc.sync.dma_start(lnb_sb[:], lnb_ext.rearrange("(f p) -> p f", p=128))

            ptiles = {}     # (pair, m) -> [128,1024] bf16 pair-tile
            bc_map = {}     # chunk -> broadcast tile
            wgq = {}        # (ot, g) -> prefetched weight tile

            def rhs(sp, q=None):
                """bf16 cos k-step sp in [0,128): pair-major, m=1..8."""
                pair, r = divmod(sp, 2 * G)
                m, sub = divmod(r, 2)
                m += 1
                if q is None:
                    return ptiles[(pair, m)][:, ds(sub * TCH, TCH)]
                return ptiles[(pair, m)][:, ds(sub * TCH + q * 128, 128)]

            def wg_get(ot, g):
                key = (ot, g)
                if key in wgq:
                    return wgq.pop(key)
                w_ = wp.tile([128, KG, 128], BF16, tag="wg")
                nc.sync.dma_start(w_[:], wt_ext[ot, g])
                return w_

            def wg_prefetch(ot, g):
                w_ = wp.tile([128, KG, 128], BF16, tag="wg")
                nc.sync.dma_start(w_[:], wt_ext[ot, g])
                wgq[(ot, g)] = w_

            def stats_steps(c):
                """Closures: 16 per-ic steps + 1 rows/broadcast step."""
                stp = statps.tile([128, TCH], F32, tag="st", name=f"st{c}")

                def ic_step(ic):
                    def f():
                        xs = xsp.tile([128, TCH], F32, tag="xs")
                        nc.gpsimd.dma_start(
                            xs[:], xt_ext[ds(ic * 128, 128), ds(c * TCH, TCH)]
                        )
                        xb = xsp.tile([128, TCH], BF16, tag="xb")
                        nc.scalar.copy(xb[:], xs[:])
                        sq = sqp.tile([128, TCH], BF16, tag="sq")
                        nc.scalar.square(sq[:], xs[:])
                        nc.tensor.matmul(stp[0:1, :], ones_bf[:], xb[:],
                                         start=(ic == 0), stop=(ic == NIC - 1))
                        nc.tensor.matmul(stp[32:33, :], ones_bf[:], sq[:],
                                         start=(ic == 0), stop=(ic == NIC - 1))
                    return f

                def rows_step():
                    mean = rowp.tile([1, TCH], F32, tag="mean", name=f"mean{c}")
                    nc.vector.tensor_scalar_mul(mean[:], stp[0:1, :], 1.0 / I)
                    var = rowp.tile([1, TCH], F32, tag="var", name=f"var{c}")
                    nc.vector.tensor_scalar_mul(var[:], stp[32:33, :], 1.0 / I)
                    row = rowp.tile([1, 2 * TCH], F32, tag="row", name=f"row{c}")
                    rr = row[0:1, 0:TCH]
                    nc.vector.tensor_tensor(rr, mean[:], mean[:], ALU.mult)
                    nc.vector.tensor_sub(var[:], var[:], rr)
                    # std in place of var, istd into the row's first half
                    nc.scalar.activation(var[:], var[:], AF.Sqrt, bias=eps_sb[:])
                    nc.vector.reciprocal(rr, var[:])
                    nc.vector.scalar_tensor_tensor(
                        row[0:1, TCH:2 * TCH], mean[:], -1.0, rr,
                        ALU.mult, ALU.mult,
                    )
                    bc = bcp.tile([128, 2 * TCH], F32, tag="bc", name=f"bc{c}")
                    nc.gpsimd.partition_broadcast(bc[:], row[0:1, :])
                    bc_map[c] = bc

                return [ic_step(ic) for ic in range(NIC)] + [rows_step]

            def gen_chunk(c, deferred=None):
                """Panel generation for chunk c; race matmuls interleaved.
                `deferred` emits the previous chunk's last o-tile drain
                after pair 0 (keeps DVE free to pre-generate pair 0)."""
                bc = bc_map.pop(c)
                iv = bc[:, 0:TCH].unsqueeze(1).broadcast_to((128, 2, TCH))
                nv = bc[:, TCH:2 * TCH].unsqueeze(1).broadcast_to((128, 2, TCH))
                pss = [
                    mmps.tile([128, TCH], F32, tag="ps", name=f"rps{r}_{c}")
                    for r in range(N_RACE)
                ]
                g_next = 0

                def race_mm(g_hi):
                    nonlocal g_next
                    for g in range(g_next, g_hi):
                        for r in range(N_RACE):
                            w_ = wg_get(r, g)
                            for ks in range(KG):
                                sp = g * KG + ks
                                nc.tensor.matmul(
                                    pss[r][:], w_[:, ks, :], rhs(sp),
                                    start=(sp == 0), stop=False,
                                )
                    g_next = g_hi

                for pair in range(NP):
                    xg = xgp.tile([128, 2 * TCH], F32, tag="xg",
                                  name=f"xg_{c}_{pair}")
                    for sub in range(2):
                        ic = 2 * pair + sub
                        nc.gpsimd.dma_start(
                            xg[:, ds(sub * TCH, TCH)],
                            xt_ext[ds(ic * 128, 128), ds(c * TCH, TCH)],
                        )
                    # normalize in place: xn = x*istd + (-mu*istd), per-token
                    nc.vector.tensor_tensor(xg[:], xg[:], iv, ALU.mult)
                    nc.vector.tensor_tensor(xg[:], xg[:], nv, ALU.add)
                    if affine:
                        for sub in range(2):
                            ic = 2 * pair + sub
                            nc.scalar.activation(
                                xg[:, ds(sub * TCH, TCH)], xg[:, ds(sub * TCH, TCH)],
                                AF.Identity,
                                bias=lnb_sb[:, ic:ic + 1], scale=lnw_sb[:, ic:ic + 1],
                            )

                    def pt(m):
                        t_ = panelp.tile([128, 2 * TCH], BF16, tag=f"p_{m}",
                                         name=f"pan_{c}_{pair}_{m}")
                        ptiles[(pair, m)] = t_
                        return t_

                    th = xgp.tile([128, 2 * TCH], F32, tag="xg",
                                  name=f"th_{c}_{pair}")
                    nc.scalar.activation(th[:], xg[:], AF.Tanh)
                    p0 = panelp.tile([128, 2, TCH], FP8E5, tag="p_0",
                                     name=f"pan_{c}_{pair}_0")
                    ptiles[(pair, 0)] = p0
                    for sub in range(2):
                        nc.scalar.activation(p0[:, sub, :], xg[:, ds(sub * TCH, TCH)],
                                             AF.Silu)
                    sh = shp.tile([128, 2 * TCH], F32, tag="sh")
                    nc.scalar.activation(sh[:], th[:], AF.Sin, scale=math.pi / 2)

                    p1, p2, p3, p4 = pt(1), pt(2), pt(3), pt(4)
                    p5, p6, p7, p8 = pt(5), pt(6), pt(7), pt(8)
                    for sub in range(2):
                        hs = ds(sub * TCH, TCH)
                        shh = sh[:, hs]

                        def scr(tag_i):
                            return scrp.tile([128, TCH], F32, tag="scr",
                                             name=f"scr{tag_i}_{c}_{pair}_{sub}")

                        def lad(tag):
                            return ladp.tile([128, TCH], F32, tag=tag,
                                             name=f"lad_{tag}_{c}_{pair}_{sub}")

                        # u = -2*sh^2 ; c1 = u + 1  (c1 on ACT)
                        u = scr("u")
                        nc.vector.scalar_tensor_tensor(u[:], shh, -2.0, shh,
                                                       ALU.mult, ALU.mult)
                        c1 = lad("c1")
                        nc.vector.tensor_scalar_add(c1[:], u[:], 1.0)
                        sq1 = scr("s1")
                        nc.scalar.square(sq1[:], c1[:])
                        c2 = lad("c2")
                        nc.vector.tensor_scalar(c2[:], sq1[:], 2.0, -1.0,
                                                ALU.mult, ALU.add)
                        u3 = scr("u3")
                        nc.vector.scalar_tensor_tensor(u3[:], c2[:], 2.0, c1[:],
                                                       ALU.mult, ALU.mult)
                        c3 = lad("c3")
                        nc.vector.tensor_sub(c3[:], u3[:], c1[:])
                        sq2 = scr("s2")
                        nc.scalar.square(sq2[:], c2[:])
                        c4 = lad("c4")
                        nc.vector.tensor_scalar(c4[:], sq2[:], 2.0, -1.0,
                                                ALU.mult, ALU.add)
                        # exports m=1..4
                        nc.scalar.copy(p1[:, hs], c1[:])
                        nc.scalar.copy(p2[:, hs], c2[:])
                        nc.scalar.copy(p3[:, hs], c3[:])
                        nc.vector.tensor_copy(p4[:, hs], c4[:])
                        # leaves m=5..8 straight to panel halves
                        u5 = scr("u5")
                        nc.vector.scalar_tensor_tensor(u5[:], c3[:], 2.0, c2[:],
                                                       ALU.mult, ALU.mult)
                        nc.vector.tensor_sub(p5[:, hs], u5[:], c1[:])
                        sq3 = scr("s3")
                        nc.scalar.square(sq3[:], c3[:])
                        nc.scalar.activation(p6[:, hs], sq3[:], AF.Identity,
                                             bias=neg1_sb[:], scale=2.0)
                        u7 = scr("u7")
                        nc.vector.scalar_tensor_tensor(u7[:], c4[:], 2.0, c3[:],
                                                       ALU.mult, ALU.mult)
                        nc.vector.tensor_sub(p7[:, hs], u7[:], c1[:])
                        sq4 = scr("s4")
                        nc.scalar.square(sq4[:], c4[:])
                        nc.scalar.activation(p8[:, hs], sq4[:], AF.Identity,
                                             bias=neg1_sb[:], scale=2.0)

                    if pair == 0 and deferred is not None:
                        deferred()
                    race_mm(2 * (pair + 1))
                race_mm(NG)
                # silu fp8 DoubleRow burst closes each race accumulation
                for r in range(N_RACE):
                    wdr = wdrp.tile([128, NP, 2, 128], FP8E5, tag="wdr",
                                    name=f"rwdr{r}_{c}")
                    nc.sync.dma_start(wdr[:], wt0_ext[r])
                    for pair in range(NP):
                        nc.tensor.matmul(
                            pss[r][:], wdr[:, pair, :, :], ptiles[(pair, 0)][:],
                            start=False, stop=(pair == NP - 1),
                            perf_mode=PM.DoubleRow, skip_group_check=True,
                        )
                for r in range(N_RACE):
                    stg = stgp.tile([128, TCH], BF16, tag="stg",
                                    name=f"rstg{r}_{c}")
                    nc.vector.tensor_copy(stg[:], pss[r][:])
                    nc.gpsimd.dma_start(
                        out_ext[ds(r * 128, 128), ds(c * TCH, TCH)], stg[:]
                    )

            def mm_chunk(c, steps):
                """O-tiles N_RACE..15; silu DR matmuls open each pass;
                next-chunk stats steps injected spread over early
                o-tiles.  Returns a deferred closure for the last
                o-tile's drain (or None)."""
                n_ots = NOT - N_RACE
                deferred = None
                for oi, ot in enumerate(range(N_RACE, NOT)):
                    last = (c == NCH - 1) and (ot == NOT - 1)
                    ps = mmps.tile([128, TCH], F32, tag="ps", name=f"mps{c}_{ot}")
                    wdr = wdrp.tile([128, NP, 2, 128], FP8E5, tag="wdr",
                                    name=f"wdr{c}_{ot}")
                    nc.sync.dma_start(wdr[:], wt0_ext[ot])
                    for pair in range(NP):
                        nc.tensor.matmul(
                            ps[:], wdr[:, pair, :, :], ptiles[(pair, 0)][:],
                            start=(pair == 0), stop=False,
                            perf_mode=PM.DoubleRow, skip_group_check=True,
                        )
                    for g in range(NG):
                        if steps and (g % 3 == 1):
                            steps.pop(0)()
                        w_ = wg_get(ot, g)
                        for ks in range(KG):
                            sp = g * KG + ks
                            if last and sp == NKB - 1:
                                for q in range(4):
                                    nc.tensor.matmul(
                                        ps[:, ds(q * 128, 128)], w_[:, ks, :],
                                        rhs(sp, q), start=False, stop=True,
                                        skip_group_check=True,
                                    )
                            else:
                                nc.tensor.matmul(
                                    ps[:], w_[:, ks, :], rhs(sp),
                                    start=False, stop=(sp == NKB - 1),
                                )

                    def drain(ps=ps, ot=ot):
                        stg = stgp.tile([128, TCH], BF16, tag="stg",
                                        name=f"stg{c}_{ot}")
                        nc.vector.tensor_copy(stg[:], ps[:])
                        nc.gpsimd.dma_start(
                            out_ext[ds(ot * 128, 128), ds(c * TCH, TCH)], stg[:]
                        )

                    if last:
                        # pipelined sliced drain to shorten the kernel tail
                        stg = stgp.tile([128, TCH], BF16, tag="stg",
                                        name=f"stg{c}_{ot}")
                        for q in range(4):
                            sl = ds(q * 128, 128)
                            nc.vector.tensor_copy(stg[:, sl], ps[:, sl])
                            nc.gpsimd.dma_start(
                                out_ext[ds(ot * 128, 128),
                                        ds(c * TCH + q * 128, 128)],
                                stg[:, sl],
                            )
                    elif oi == n_ots - 1 and c + 1 < NCH:
                        deferred = drain
                    else:
                        drain()
                while steps:
                    steps.pop(0)()
                return deferred

            # --- program ---
            for g in range(2):
                for r in range(N_RACE):
                    if len(wgq) < 7:
                        wg_prefetch(r, g)
            for f in stats_steps(0):
                f()
            deferred = None
            for c in range(NCH):
                gen_chunk(c, deferred=deferred)
                steps = stats_steps(c + 1) if c + 1 < NCH else []
                deferred = mm_chunk(c, steps)

    nc.compile()
    return nc


def prep_weights(base_weight, spline_weight):
    """Host-side weight prep.
    wt  (bf16 cos part): k-step sp = pair*16 + (m-1)*2 + sub, tiled
        [ot, g, k_in, ks, o_in] for [128, KG, 128] DMAs.
    wt0 (fp8e5 silu part, DoubleRow): [ot, k_in, pair, sub, o_in]."""
    w = np.empty((G, I, O), np.float32)
    for g in range(G):
        w[g] = spline_weight[:, :, g].T       # [i, o]
    w = w.reshape(G, NP, 2, 128, O).transpose(1, 0, 2, 3, 4)  # [pair, m-1, sub, 128, o]
    w = w.reshape(NKB * 128, O)
    w = w.reshape(NG, KG, 128, NOT, 128).transpose(3, 0, 2, 1, 4)
    wt = np.ascontiguousarray(w.astype(ml_dtypes.bfloat16))
    wb = base_weight.T.reshape(NP, 2, 128, NOT, 128)  # [pair, sub, kin, ot, oin]
    wb = wb.transpose(3, 2, 0, 1, 4)                  # [ot, kin, pair, sub, oin]
    wt0 = np.ascontiguousarray(wb.astype(ml_dtypes.float8_e5m2))
    return wt, wt0


_NC_CACHE = {}


def _get_nc(affine=False):
    if affine not in _NC_CACHE:
        _NC_CACHE[affine] = build_nc(affine=affine)
    return _NC_CACHE[affine]


def kernel(x, ln_weight, ln_bias, base_weight, spline_weight):
    x = np.asarray(x, np.float32)
    ln_weight = np.asarray(ln_weight, np.float32)
    ln_bias = np.asarray(ln_bias, np.float32)
    affine = not (np.all(ln_weight == 1.0) and np.all(ln_bias == 0.0))
    wt, wt0 = prep_weights(np.asarray(base_weight, np.float32),
                           np.asarray(spline_weight, np.float32))
    nc = _get_nc(affine)
    in_maps = [
        {
            "xt": np.ascontiguousarray(x[b].T),
            "lnw": ln_weight,
            "lnb": ln_bias,
            "wt": wt,
            "wt0": wt0,
        }
        for b in range(B)
    ]
    res = run_bass_kernel_spmd(nc, in_maps, core_ids=list(range(B)))
    out = np.stack([res.results[b]["out"].astype(np.float32).T for b in range(B)])
    return np.ascontiguousarray(out)
